# revision 8
# baseline (speedup 1.0000x reference)
"""Trainium2 Bass kernel for nn_Fractal1D (soft fractal / smoothed decision-tree descent).

Reference computation (per point x, N=131072 points, M=128 nodes, depth 10):
    split = sigmoid(4*p - 2); values = tile(3*v + 1, 4)
    w0 = e_0;  lo=0, hi=1
    repeat 10x:
        s  = lo + (w @ split) * (hi - lo)
        t  = sigmoid((x - s) / 0.1)
        w  = (1-t) * (w @ L) + t * (w @ R)
        lo, hi = (1-t)*lo + t*s, (1-t)*s + t*hi
    out = w @ values

Key observation: y(x) is a scalar function of scalar x (all other inputs are
shared parameters), and with smoothing width 0.1 it is very smooth (max |y'|
~0.6, range ~0.1).  Piecewise-linear interpolation on a 128-knot grid
reproduces it to ~1e-4 absolute (tolerance is 2e-2 relative on scale ~2.5).

Kernel strategy (data-parallel over 8 cores, 16384 points/core):
  1. Knot evaluation: run the full fractal recursion once on a single
     [128 nodes x 128 knots] tile (knots at k/127; pure constants, so this
     phase has no DMA dependency and starts immediately).
       - matvec sdot = split^T w uses a rank-1 lhsT (split x ones) so the
         result lands REPLICATED across all 128 partitions; the whole row
         state (xml = x - lo, dd = hi - lo) is kept replicated, eliminating
         any broadcast step.
       - blend restructured as w' = Lw + t*( (R-L) w ): both matmuls depend
         only on w (not t), so they run off the critical chain; the chain is
         sdot -> g,xms (DVE) -> sigmoid (ACT) -> m1,w' (DVE) -> next sdot.
       - interval updates (xml' = xml - t*g, dd' = g + t*(dd - 2g)) on gpsimd,
         off the chain.
     Final: T[k] = (values-2.5).w10 via a plain-f32 matvec (exact), slope
     dT[k] = T[k+1]-T[k] via a shift matmul.
  2. Interpolation of the 16384 points (32 chunks of F=512):
       - idx = floor(x*127) and frac via AluOp mod in row layout [32, 512];
       - per chunk: idx broadcast to 128 partitions with one esel matmul,
         one-hot = is_equal(idx_bcast, partition) into bf16;
       - one stacked gather matmul per chunk: lhsT block [128, 64] holding
         Tdelta in column i and dT in column 32+i accumulates into a single
         [64, 512] PSUM tile (rows 0-31 = Tdelta[idx] per chunk, rows 32-63 =
         dT[idx]); one extra rank-1 matmul (2.5 * column-sum of a one-hot)
         adds the subtracted mean back;
       - y = (2.5 + Tdelta[idx]) + frac * dT[idx], DMA out.
     The idx/one-hot prep is independent of the knot values, so it is
     interleaved under the knot-eval critical chain (2 chunks per depth);
     the rest drains during the gather phase.
  T and one-hots use bf16 operands for the gather matmuls; T is mean-shifted
  (+-0.06 range) so bf16 rounding costs ~2e-4 absolute.
"""

from contextlib import ExitStack

import numpy as np

import concourse.bacc as bacc
import concourse.bass as bass
import concourse.tile as tile
from concourse import mybir
from concourse.bass_utils import run_bass_kernel_spmd

F32 = mybir.dt.float32
F32R = mybir.dt.float32r
BF16 = mybir.dt.bfloat16
I16 = mybir.dt.int16
AOP = mybir.AluOpType
AFT = mybir.ActivationFunctionType

N_TOTAL = 131072
NCORES = 8
NPTS = N_TOTAL // NCORES      # 16384 points per core
F = 512                       # points per interp chunk
NCH = NPTS // F               # 32 chunks
NROW = NCH                    # row-layout partitions for point math
M = 128                       # fractal nodes
K = 128                       # interpolation knots (127 intervals)
DEPTH = 10
INV_SMOOTH = 10.0             # 1 / smoothing_width
YMEAN = 2.5                   # mean shift for bf16 gather precision
CLAMP = 127.0 - 2.0 ** -11    # idx stays <= 126 for any x <= 1.0


def f32(ap):
    """View an f32r/bf16-declared AP as plain fp32 where bit-identical."""
    return ap.bitcast(F32)


def _emit(nc, bench_reps=1):
    x_in = nc.declare_dram_parameter("x", [NPTS], F32, isOutput=False)
    spp_in = nc.declare_dram_parameter("spp", [M], F32, isOutput=False)
    vp_in = nc.declare_dram_parameter("vp", [32], F32, isOutput=False)
    l_in = nc.declare_dram_parameter("lmat", [M, M], F32, isOutput=False)
    r_in = nc.declare_dram_parameter("rmat", [M, M], F32, isOutput=False)
    y_out = nc.declare_dram_parameter("y", [NPTS], F32, isOutput=True)

    with tile.TileContext(nc) as tc, ExitStack() as ctx:
        sing = ctx.enter_context(tc.tile_pool(name="sing", bufs=1))
        scratch = ctx.enter_context(tc.tile_pool(name="scratch", bufs=2))
        tpool = ctx.enter_context(tc.tile_pool(name="tpool", bufs=2))
        ps_ib = ctx.enter_context(tc.tile_pool(name="ps_ib", bufs=3, space="PSUM"))
        ps_sdot = ctx.enter_context(tc.tile_pool(name="ps_sdot", bufs=1, space="PSUM"))
        ps_lw = ctx.enter_context(tc.tile_pool(name="ps_lw", bufs=1, space="PSUM"))
        ps_d = ctx.enter_context(tc.tile_pool(name="ps_d", bufs=1, space="PSUM"))
        ps_g = ctx.enter_context(tc.tile_pool(name="ps_g", bufs=1, space="PSUM"))

        # ---------------- constants / parameter transforms ----------------
        l_sb = sing.tile([M, M], F32, tag="l_sb")
        r_sb = sing.tile([M, M], F32, tag="r_sb")
        nc.sync.dma_start(out=l_sb, in_=l_in[:, :])
        nc.sync.dma_start(out=r_sb, in_=r_in[:, :])
        l_r = sing.tile([M, M], F32R, tag="l_r")
        nc.scalar.copy(l_r, l_sb)
        rml = sing.tile([M, M], F32R, tag="rml")
        nc.vector.tensor_sub(rml, r_sb, l_sb)

        spp_sb = sing.tile([M, 1], F32, tag="spp_sb")
        nc.sync.dma_start(out=spp_sb, in_=spp_in[:].rearrange("(p f) -> p f", f=1))
        spp_pre = sing.tile([M, 1], F32, tag="spp_pre")
        nc.vector.tensor_scalar(spp_pre, spp_sb, 4.0, -2.0, op0=AOP.mult, op1=AOP.add)
        split_sb = sing.tile([M, 1], F32, tag="split_sb")
        nc.scalar.activation(split_sb, spp_pre, AFT.Sigmoid)

        # splitbc[p, i] = split[p] for all i (rank-1 lhsT -> replicated matvec)
        ones_mm = sing.tile([M, M], F32, tag="ones_mm")
        nc.vector.memset(ones_mm, 1.0)
        splitbc = sing.tile([M, M], F32R, tag="splitbc")
        nc.vector.tensor_scalar(splitbc, ones_mm, split_sb, None, op0=AOP.mult)

        # values (mean-shifted): vd128 = 3*tile(vp,4) + 1 - YMEAN
        vd128 = sing.tile([M, 1], F32, tag="vd128")
        vp_ap = vp_in[:]
        vp_bcast = bass.AP(tensor=vp_ap.tensor, offset=vp_ap.offset, ap=[[0, 4], [1, 32]])
        nc.sync.dma_start(out=vd128, in_=vp_bcast)
        nc.vector.tensor_scalar(
            vd128, vd128, 3.0, 1.0 - YMEAN, op0=AOP.mult, op1=AOP.add
        )

        # knot x values replicated on every partition: xk_rep[p, c] = c/127
        with tc.tile_pool(name="setup", bufs=1) as setup:
            iot_k = setup.tile([M, K], I16, tag="iot_k")
            nc.gpsimd.iota(iot_k, pattern=[[1, K]], base=0, channel_multiplier=0)
            xk_rep = sing.tile([M, K], F32, tag="xk_rep")
            nc.vector.tensor_scalar(xk_rep, iot_k, 1.0 / (K - 1.0), None, op0=AOP.mult)

            # shiftmat[p, c] = (c == p-1): lhsT for T[i+1] shift matvec
            iot_s = setup.tile([M, M], I16, tag="iot_s")
            nc.gpsimd.iota(iot_s, pattern=[[1, M]], base=1, channel_multiplier=-1)
            shiftmat = sing.tile([M, M], F32, tag="shiftmat")
            nc.vector.tensor_scalar(shiftmat, iot_s, 0, None, op0=AOP.is_equal)

            # esel32[q, i, p] = (q == i): idx-row broadcast lhsT blocks
            iot_e = setup.tile([NROW, NCH, M], I16, tag="iot_e")
            nc.gpsimd.iota(
                iot_e, pattern=[[1, NCH], [0, M]], base=0, channel_multiplier=-1
            )
            esel32 = sing.tile([NROW, NCH, M], F32R, tag="esel32")
            nc.vector.tensor_scalar(esel32, iot_e, 0, None, op0=AOP.is_equal)

            # maskC[p, i, j] = (j == i): TE diagonal placement mask
            iot_m = setup.tile([M, NCH, NCH], I16, tag="iot_m")
            nc.gpsimd.iota(
                iot_m, pattern=[[-1, NCH], [1, NCH]], base=0, channel_multiplier=0
            )
            maskC = sing.tile([M, NCH, NCH], BF16, tag="maskC")
            nc.vector.tensor_scalar(maskC, iot_m, 0, None, op0=AOP.is_equal)

            # c25[p, r] = YMEAN for r < 32 else 0 (mean add-back via matmul)
            iot_c = setup.tile([M, 2 * NCH], I16, tag="iot_c")
            nc.gpsimd.iota(iot_c, pattern=[[1, 2 * NCH]], base=0, channel_multiplier=0)
            c25 = sing.tile([M, 2 * NCH], BF16, tag="c25")
            nc.vector.tensor_scalar(
                c25, iot_c, NCH, YMEAN, op0=AOP.is_lt, op1=AOP.mult
            )

            # iota_f32[p] = p (knot id per partition, for the one-hot compare)
            iot_p = setup.tile([M, 1], I16, tag="iot_p")
            nc.gpsimd.iota(iot_p, pattern=[[1, 1]], base=0, channel_multiplier=1)
            iota_f32 = sing.tile([M, 1], F32, tag="iota_f32")
            nc.vector.tensor_scalar(iota_f32, iot_p, 0, None, op0=AOP.add)

        # depth-0 constants: w0 = e_0 so everything depends on split[0] only
        l0col = sing.tile([M, 1], F32, tag="l0col")
        nc.sync.dma_start(out=l0col, in_=l_in[0, :].rearrange("(p f) -> p f", f=1))
        r0col = sing.tile([M, 1], F32, tag="r0col")
        nc.sync.dma_start(out=r0col, in_=r_in[0, :].rearrange("(p f) -> p f", f=1))
        rml0 = sing.tile([M, 1], F32, tag="rml0")
        nc.vector.tensor_sub(rml0, r0col, l0col)

        spp0 = sing.tile([M, 1], F32, tag="spp0")
        spp_ap = spp_in[:]
        spp0_bc = bass.AP(tensor=spp_ap.tensor, offset=spp_ap.offset, ap=[[0, M], [1, 1]])
        nc.sync.dma_start(out=spp0, in_=spp0_bc)
        s0col = sing.tile([M, 1], F32, tag="s0col")
        nc.vector.tensor_scalar(s0col, spp0, 4.0, -2.0, op0=AOP.mult, op1=AOP.add)
        nc.scalar.activation(s0col, s0col, AFT.Sigmoid)
        b0col = sing.tile([M, 1], F32, tag="b0col")       # -10*s0 (sigmoid bias)
        nc.vector.tensor_scalar_mul(b0col, s0col, -INV_SMOOTH)
        negs0 = sing.tile([M, 1], F32, tag="negs0")       # -s0
        nc.vector.tensor_scalar_mul(negs0, s0col, -1.0)
        oneM2s0 = sing.tile([M, 1], F32, tag="oneM2s0")   # 1 - 2*s0
        nc.vector.tensor_scalar(oneM2s0, s0col, -2.0, 1.0, op0=AOP.mult, op1=AOP.add)

        # ---------------- persistent state ----------------
        w_bufs = [
            sing.tile([M, K], F32R, tag="w_ping", name="w_ping"),
            sing.tile([M, K], F32R, tag="w_pong", name="w_pong"),
        ]
        w10_f32 = sing.tile([M, K], F32, tag="w10_f32")
        xml = sing.tile([M, K], F32, tag="xml")
        dd = sing.tile([M, K], F32, tag="dd")
        oh_all = sing.tile([M, NCH, F], BF16, tag="oh_all")
        TEBOTH = sing.tile([M, NCH, 2 * NCH], BF16, tag="TEBOTH")
        nc.vector.memset(TEBOTH, 0.0)  # off-diagonal cols stay 0 forever

        xrow = sing.tile([NROW, F], F32, tag="xrow")
        xkrow = sing.tile([NROW, F], F32, tag="xkrow")
        magic = sing.tile([NROW, F], F32, tag="magic")
        frac = sing.tile([NROW, F], F32, tag="frac")
        idxf = sing.tile([NROW, F], F32R, tag="idxf")
        fd_sb = sing.tile([NROW, F], F32, tag="fd_sb")
        ysb = sing.tile([NROW, F], F32, tag="ysb")
        Tcol = sing.tile([M, 1], F32, tag="Tcol")
        dTcol = sing.tile([M, 1], F32, tag="dTcol")

        def body():
            # ---- input DMA + row prep (independent of knot values) ----
            nc.sync.dma_start(
                out=xrow, in_=x_in[:].rearrange("(p f) -> p f", f=F)
            )
            nc.vector.tensor_scalar(
                xkrow, xrow, K - 1.0, CLAMP, op0=AOP.mult, op1=AOP.min
            )
            # idx = round(min(xk, 126.4999)) via the 2^23 magic-number trick
            # (AluOp mod fails the hw ISA check).  frac = xk - idx lands in
            # [-0.5, 1): negative frac uses the adjacent interval's slope,
            # a ~5e-4 absolute error; the top interval interpolates forward.
            nc.vector.tensor_scalar(
                magic, xkrow, K - 1.5 - 0.0001, 8388608.0, op0=AOP.min, op1=AOP.add
            )
            nc.vector.tensor_scalar(idxf, magic, 8388608.0, None, op0=AOP.subtract)
            nc.vector.tensor_sub(frac, xkrow, f32(idxf))

            ib_ps = [None] * NCH

            def emit_bcast(i):
                """PE: replicate idx row i across 128 partitions."""
                ib = ps_ib.tile([M, F], F32, tag="ib", name=f"ib{i}")
                ib_ps[i] = ib
                nc.tensor.matmul(
                    ib, lhsT=esel32[:, i, :], rhs=idxf, start=True, stop=True
                )

            def emit_iseq(i):
                """DVE: one-hot of idx against the partition id."""
                nc.vector.tensor_scalar(
                    oh_all[:, i, :], ib_ps[i], iota_f32, None, op0=AOP.is_equal
                )

            # ---- depth 0 (constants only; starts the chain immediately) ----
            t0 = tpool.tile([M, K], F32R, tag="t", name="t0")
            nc.scalar.activation(t0, xk_rep, AFT.Sigmoid, bias=b0col, scale=INV_SMOOTH)
            nc.vector.tensor_scalar(
                w_bufs[1], f32(t0), rml0, l0col, op0=AOP.mult, op1=AOP.add
            )
            tg0 = scratch.tile([M, K], F32, tag="tg", name="tg0")
            nc.gpsimd.tensor_scalar(tg0, f32(t0), negs0, None, op0=AOP.mult)
            nc.gpsimd.tensor_add(xml, tg0, xk_rep)
            nc.gpsimd.tensor_scalar(
                dd, f32(t0), oneM2s0, s0col, op0=AOP.mult, op1=AOP.add
            )

            chunk = [0]

            def emit_pair():
                if chunk[0] < NCH:
                    emit_bcast(chunk[0])
                    emit_iseq(chunk[0])
                    chunk[0] += 1

            emit_pair()
            emit_pair()

            # ---- depths 1..9 ----
            for d in range(1, DEPTH):
                w_cur = w_bufs[d % 2]
                last = d == DEPTH - 1
                w_next = w10_f32 if last else w_bufs[(d + 1) % 2]

                sdot = ps_sdot.tile([M, K], F32, tag="sdot", name=f"sdot{d}")
                nc.tensor.matmul(sdot, lhsT=splitbc, rhs=w_cur, start=True, stop=True)
                lw = ps_lw.tile([M, K], F32, tag="lw", name=f"lw{d}")
                nc.tensor.matmul(lw, lhsT=l_r, rhs=w_cur, start=True, stop=True)
                dps = ps_d.tile([M, K], F32, tag="dps", name=f"dps{d}")
                nc.tensor.matmul(dps, lhsT=rml, rhs=w_cur, start=True, stop=True)
                if chunk[0] < NCH:
                    emit_bcast(chunk[0])
                if chunk[0] + 1 < NCH:
                    emit_bcast(chunk[0] + 1)

                g_sb = scratch.tile([M, K], F32, tag="g", name=f"g{d}")
                nc.vector.tensor_mul(g_sb, sdot, dd)
                xms = scratch.tile([M, K], F32, tag="xms", name=f"xms{d}")
                nc.vector.tensor_sub(xms, xml, g_sb)
                if chunk[0] < NCH:
                    emit_iseq(chunk[0])

                tg_t = tpool.tile([M, K], F32R, tag="t", name=f"t{d}")
                nc.scalar.activation(tg_t, xms, AFT.Sigmoid, scale=INV_SMOOTH)

                m1 = scratch.tile([M, K], F32, tag="m1", name=f"m1{d}")
                nc.vector.tensor_mul(m1, f32(tg_t), dps)
                nc.vector.tensor_add(w_next, m1, lw)
                if chunk[0] + 1 < NCH:
                    emit_iseq(chunk[0] + 1)
                chunk[0] = min(chunk[0] + 2, NCH)

                if not last:
                    # interval updates (off-chain, gpsimd)
                    tg = scratch.tile([M, K], F32, tag="tg", name=f"tg{d}")
                    nc.gpsimd.tensor_mul(tg, f32(tg_t), g_sb)
                    nc.gpsimd.tensor_sub(xml, xml, tg)
                    u_sb = scratch.tile([M, K], F32, tag="u", name=f"u{d}")
                    nc.gpsimd.tensor_scalar(u_sb, g_sb, -2.0, None, op0=AOP.mult)
                    nc.gpsimd.tensor_add(u_sb, u_sb, dd)
                    v_sb = scratch.tile([M, K], F32, tag="v", name=f"v{d}")
                    nc.gpsimd.tensor_mul(v_sb, f32(tg_t), u_sb)
                    nc.gpsimd.tensor_add(dd, v_sb, g_sb)

            # ---- knot table: T = (values-2.5) . w10  (plain f32, exact) ----
            T_ps = ps_sdot.tile([M, K], F32, tag="sdot", name="T_ps")
            nc.tensor.matmul(
                T_ps[:, 0:1], lhsT=w10_f32, rhs=vd128, start=True, stop=True
            )
            nc.scalar.copy(Tcol, T_ps[:, 0:1])
            Tsh_ps = ps_lw.tile([M, K], F32, tag="lw", name="Tsh_ps")
            nc.tensor.matmul(
                Tsh_ps[:, 0:1], lhsT=shiftmat, rhs=Tcol, start=True, stop=True
            )
            nc.vector.tensor_sub(dTcol, Tsh_ps[:, 0:1], Tcol)

            # TE blocks: col i -> Tdelta, col 32+i -> dT (disjoint writes)
            nc.vector.tensor_scalar(
                TEBOTH[:, :, 0:NCH], maskC, Tcol, None, op0=AOP.mult
            )
            nc.gpsimd.tensor_scalar(
                TEBOTH[:, :, NCH : 2 * NCH], maskC, dTcol, None, op0=AOP.mult
            )

            # ---- gather phase: drain remaining one-hots + 33 matmuls ----
            while chunk[0] < NCH:
                emit_pair()

            g_ps = ps_g.tile([2 * NCH, F], F32, tag="gather", name="g_ps")
            nc.tensor.matmul(
                g_ps, lhsT=c25, rhs=oh_all[:, 0, :], start=True, stop=False
            )
            for i in range(NCH):
                nc.tensor.matmul(
                    g_ps,
                    lhsT=TEBOTH[:, i, :],
                    rhs=oh_all[:, i, :],
                    start=False,
                    stop=(i == NCH - 1),
                )

            # ---- y = (2.5 + Tdelta[idx]) + frac * dT[idx] ----
            nc.vector.tensor_mul(fd_sb, frac, g_ps[NCH : 2 * NCH, :])
            nc.vector.tensor_add(ysb, fd_sb, g_ps[0:NCH, :])
            nc.sync.dma_start(
                out=y_out[:].rearrange("(p f) -> p f", f=F), in_=ysb
            )

        if bench_reps > 1:
            with tc.For_i(0, bench_reps, 1):
                body()
        else:
            body()

    return nc


_CACHE = {}


def build_bench(reps):
    """Fresh module with the whole computation repeated `reps` times on-device."""
    nc = bacc.Bacc("TRN2", target_bir_lowering=False)
    _emit(nc, bench_reps=reps)
    nc.compile()
    return nc


def build_bass(compiled=True):
    """Build (and by default finalize) the Bacc module.

    compiled=False returns the pre-compile module for CoreSim runs.
    """
    if "nc" not in _CACHE:
        nc = bacc.Bacc("TRN2", target_bir_lowering=False)
        _emit(nc)
        _CACHE["nc"] = nc
    nc = _CACHE["nc"]
    if compiled and not _CACHE.get("compiled"):
        nc.compile()
        _CACHE["compiled"] = True
    return nc


def make_in_maps(x, split_points_param, values_param, left_matrix, right_matrix):
    x = np.ascontiguousarray(x, dtype=np.float32)
    shards = x.reshape(NCORES, NPTS)
    common = {
        "spp": np.ascontiguousarray(split_points_param, dtype=np.float32),
        "vp": np.ascontiguousarray(values_param, dtype=np.float32),
        "lmat": np.ascontiguousarray(left_matrix, dtype=np.float32),
        "rmat": np.ascontiguousarray(right_matrix, dtype=np.float32),
    }
    return [{"x": shards[i], **common} for i in range(NCORES)]


def kernel(x, split_points_param, values_param, left_matrix, right_matrix, max_depth):
    assert int(max_depth) == DEPTH
    nc = build_bass()
    in_maps = make_in_maps(
        x, split_points_param, values_param, left_matrix, right_matrix
    )
    res = run_bass_kernel_spmd(nc, in_maps, list(range(NCORES)))
    out = np.concatenate([res.results[i]["y"] for i in range(NCORES)])
    return out.astype(np.float32)
